# revision 28
# baseline (speedup 1.0000x reference)
"""AdaFace loss on 8 TRN2 NeuronCores, class-parallel.

Strategy: shard the 100k weight rows (classes) across 8 cores. Host
pre-normalizes rows, transposes to [D, C_shard], scales by 8 and casts to
fp8e4 (scale keeps values out of the e4m3 subnormal range; the ScalarE
exp absorbs it: exp(0.5*x - 32) of the 64*cos matmul result). Since
|logit| <= 32, a fixed shift of 32 replaces the per-row max of a
standard log-softmax, so no max collective is needed. Each core returns
per-batch-chunk partial sums of exp(32c-32); the host does the final
O(B) combine: sum across cores, margin-target correction (cos(theta+m)
needs only sqrt, no arccos), ln, weighted dot. No device collective.

Device per core: the whole fp8 weight shard stays resident in SBUF
(24.5KB/partition); DoubleRow matmuls (K=256 per instruction) fill
4-bank PSUM tiles [128b, 2048c]; ScalarE exp -> bf16; VectorE 2x-mode
running adds + one final reduce per batch chunk; single DMA out.
"""

import numpy as np
import ml_dtypes

import concourse.bass as bass
import concourse.tile as tile
from concourse import bacc, mybir
from concourse.bass_utils import run_bass_kernel_spmd

B = 512
D = 256
C = 100000
NCORES = 8
CSH = C // NCORES          # 12500 classes per core
# class tiles per core: six 2048-wide + one 256-wide = 12544.
# Most batch chunks put the 256-wide tile first (cheap pipeline starter);
# the last chunk puts it last so the wide columns can reduce early.
TILES_SMALL_FIRST = [(12288, 256)] + [(i * 2048, 2048) for i in range(6)]
TILES_SMALL_LAST = [(i * 2048, 2048) for i in range(6)] + [(12288, 256)]
CPAD = 12544
NPAD_TOT = (CPAD - CSH) * NCORES

M0 = 0.5
M_MIN = 0.25
SCALE = 32.0
SHIFT = 32.0               # fixed log-softmax shift (|logits| <= SCALE)
FP8_PRESCALE = 8.0         # both operands scaled by 8 -> matmul gives 64*cos

f32 = mybir.dt.float32
bf16 = mybir.dt.bfloat16
fp8 = mybir.dt.float8e4

NBC = B // 128             # 4 batch chunks

_cached_nc = None
_last_results = None


def _build():
    global _cached_nc
    if _cached_nc is not None:
        return _cached_nc

    nc = bacc.Bacc(
        "TRN2", target_bir_lowering=False, debug=False, num_devices=NCORES
    )

    # [p, j, c] with contraction index k = j*128 + p
    wnT_d = nc.dram_tensor("wnT", [128, 2, CPAD], fp8, kind="ExternalInput")
    featnT_d = nc.dram_tensor("featnT", [128, 2, B], fp8, kind="ExternalInput")
    out_d = nc.dram_tensor("out", [128, NBC], f32, kind="ExternalOutput")

    with tile.TileContext(nc) as tc:
        with (
            tc.tile_pool(name="persist", bufs=1) as persist,
            tc.tile_pool(name="epool", bufs=5) as epool,
            tc.tile_pool(name="psum", bufs=2, space="PSUM") as psum,
        ):
            fsb = persist.tile([128, 2, B], fp8)
            nc.sync.dma_start(out=fsb[:], in_=featnT_d[:])

            wsb = persist.tile([128, 2, CPAD], fp8)
            # chunked loads in consumption order: the small tile's columns
            # first (gates the pipeline start), then the wide region.
            # Spread across the three DMA-capable engines' queues — a single
            # queue serializes at ~50GB/s and trickles for the whole kernel.
            # (engine, lo, hi) in consumption order across the two HWDGE
            # queues (gpsimd's SWDGE path is too slow for bulk loads)
            plan = [
                (nc.scalar, 12288, 12544),
                (nc.scalar, 0, 1536),
                (nc.sync, 1536, 3072),
                (nc.scalar, 3072, 4608),
                (nc.sync, 4608, 6144),
                (nc.scalar, 6144, 7680),
                (nc.sync, 7680, 9216),
                (nc.scalar, 9216, 10752),
                (nc.gpsimd, 10752, 12288),
            ]
            for eng, lo, hi in plan:
                eng.dma_start(
                    out=wsb[:, :, lo:hi], in_=wnT_d[:, :, lo:hi]
                )

            bias_s = persist.tile([128, 1], f32)
            nc.gpsimd.memset(bias_s[:], -SHIFT)

            eacc = [
                persist.tile(
                    [128, 2048], bf16, tag=f"eacc{bc}", name=f"eacc{bc}"
                )
                for bc in range(NBC)
            ]

            S_all = persist.tile([128, NBC], f32)

            S_main = persist.tile([128, 1], f32)

            for bc in range(NBC):
                last = bc == NBC - 1
                tiles = TILES_SMALL_LAST if last else TILES_SMALL_FIRST
                lhs = fsb[:, :, bc * 128:(bc + 1) * 128]
                for ti, (c0, cw) in enumerate(tiles):
                    ps = psum.tile([128, 2048], f32, tag="ps")
                    for j in range(0, cw, 512):
                        jw = min(512, cw - j)
                        nc.tensor.matmul(
                            ps[:, j:j + jw],
                            lhs,
                            wsb[:, :, c0 + j:c0 + j + jw],
                            start=True, stop=True,
                            perf_mode=mybir.MatmulPerfMode.DoubleRow,
                        )
                    esc = epool.tile([128, 2048], bf16, tag="esc")
                    nc.scalar.activation(
                        esc[:, :cw], ps[:, :cw],
                        mybir.ActivationFunctionType.Exp,
                        bias=bias_s[:], scale=SCALE / (FP8_PRESCALE**2),
                    )
                    if ti == 0:
                        nc.vector.tensor_copy(
                            eacc[bc][:, :cw], esc[:, :cw]
                        )
                    elif ti == 1 and not last:
                        nc.vector.tensor_add(
                            eacc[bc][:, :256], eacc[bc][:, :256], esc[:, :256]
                        )
                        nc.vector.tensor_copy(
                            eacc[bc][:, 256:], esc[:, 256:]
                        )
                    else:
                        nc.vector.tensor_add(
                            eacc[bc][:, :cw], eacc[bc][:, :cw], esc[:, :cw]
                        )
                    if bc > 0 and ti == 1:
                        nc.vector.tensor_reduce(
                            S_all[:, bc - 1:bc],
                            eacc[bc - 1][:],
                            axis=mybir.AxisListType.X,
                            op=mybir.AluOpType.add,
                        )
                    if last and ti == len(tiles) - 2:
                        nc.vector.tensor_reduce(
                            S_main[:],
                            eacc[bc][:, 256:2048],
                            axis=mybir.AxisListType.X,
                            op=mybir.AluOpType.add,
                        )

            S_small = persist.tile([128, 1], f32)
            nc.vector.tensor_reduce(
                S_small[:],
                eacc[NBC - 1][:, 0:256],
                axis=mybir.AxisListType.X,
                op=mybir.AluOpType.add,
            )
            nc.vector.tensor_add(S_all[:, NBC - 1:NBC], S_main[:], S_small[:])

            nc.sync.dma_start(out=out_d[:], in_=S_all[:])

    nc.compile()
    _cached_nc = nc
    return nc


def _host_prep(features, weight, weights, labels):
    """Everything O(B*D) / O(C*D) that is not the big matmul."""
    f = features.astype(np.float64)
    norms = np.sqrt((f * f).sum(axis=1))
    lo, hi = norms.min(), norms.max()
    denom = max(hi - lo, 1e-8)
    margins = np.clip(M_MIN + (M0 - M_MIN) * (norms - lo) / denom, M_MIN, M0)
    feat_n = f / np.maximum(norms, 1e-12)[:, None]

    wlab = weight[labels].astype(np.float64)
    wlab_n = wlab / np.maximum(
        np.sqrt((wlab * wlab).sum(axis=1)), 1e-12
    )[:, None]
    cos_t = np.clip((feat_n * wlab_n).sum(axis=1), -1.0 + 1e-7, 1.0 - 1e-7)
    cos_m = cos_t * np.cos(margins) - np.sqrt(1.0 - cos_t * cos_t) * np.sin(
        margins
    )
    t_logit = SCALE * cos_m
    corr = (
        np.exp(SCALE * cos_m - SHIFT)
        - np.exp(SCALE * cos_t - SHIFT)
        - NPAD_TOT * np.exp(-SHIFT)
    )
    coef = weights.astype(np.float64) / B
    return feat_n, corr, coef, t_logit


def _to_dr_layout(mat_t, width):
    """[D, X] f32 -> [128, 2, X] fp8 with k = j*128 + p."""
    a = mat_t.reshape(2, 128, width)          # [j, p, X]
    a = np.ascontiguousarray(a.transpose(1, 0, 2))  # [p, j, X]
    return a.astype(ml_dtypes.float8_e4m3)


def kernel(features, weight, weights, labels):
    global _last_results
    features = np.asarray(features, dtype=np.float32)
    weight = np.asarray(weight, dtype=np.float32)
    weights = np.asarray(weights, dtype=np.float32)
    labels = np.asarray(labels).astype(np.int64)

    feat_n, corr, coef, t_logit = _host_prep(features, weight, weights, labels)

    wn = weight / np.maximum(
        np.linalg.norm(weight, axis=1, keepdims=True), 1e-12
    )
    featnT = np.ascontiguousarray(feat_n.T.astype(np.float32)) * FP8_PRESCALE
    featnT8 = _to_dr_layout(featnT, B)

    in_maps = []
    for i in range(NCORES):
        sh = wn[i * CSH:(i + 1) * CSH]  # [CSH, D]
        wt = np.zeros((D, CPAD), dtype=np.float32)
        wt[:, :CSH] = sh.T * FP8_PRESCALE
        in_maps.append(
            {"wnT": _to_dr_layout(wt, CPAD), "featnT": featnT8}
        )

    nc = _build()
    res = run_bass_kernel_spmd(nc, in_maps, list(range(NCORES)))
    _last_results = res

    # ---- host combine ----
    S = np.zeros(B, dtype=np.float64)
    for i in range(NCORES):
        sc = np.asarray(res.results[i]["out"], dtype=np.float64)  # [128, 4]
        for bc in range(NBC):
            S[bc * 128:(bc + 1) * 128] += sc[:, bc]

    Z = S + corr
    per = SHIFT + np.log(Z) - t_logit
    loss = float((coef * per).sum())
    return np.array(loss, dtype=np.float32)


# revision 29
# speedup vs baseline: 1.0375x; 1.0375x over previous
"""AdaFace loss on 8 TRN2 NeuronCores, class-parallel.

Strategy: shard the 100k weight rows (classes) across 8 cores. Host
pre-normalizes rows, transposes to [D, C_shard], scales by 8 and casts to
fp8e4 (scale keeps values out of the e4m3 subnormal range; the ScalarE
exp absorbs it: exp(0.5*x - 32) of the 64*cos matmul result). Since
|logit| <= 32, a fixed shift of 32 replaces the per-row max of a
standard log-softmax, so no max collective is needed. Each core returns
per-batch-chunk partial sums of exp(32c-32); the host does the final
O(B) combine: sum across cores, margin-target correction (cos(theta+m)
needs only sqrt, no arccos), ln, weighted dot. No device collective.

Device per core: the whole fp8 weight shard stays resident in SBUF
(24.5KB/partition); DoubleRow matmuls (K=256 per instruction) fill
4-bank PSUM tiles [128b, 2048c]; ScalarE exp -> bf16; VectorE 2x-mode
running adds + one final reduce per batch chunk; single DMA out.
"""

import numpy as np
import ml_dtypes

import concourse.bass as bass
import concourse.tile as tile
from concourse import bacc, mybir
from concourse.bass_utils import run_bass_kernel_spmd

B = 512
D = 256
C = 100000
NCORES = 8
CSH = C // NCORES          # 12500 classes per core
# class tiles per core: six 2048-wide + one 256-wide = 12544.
# Most batch chunks put the 256-wide tile first (cheap pipeline starter);
# the last chunk puts it last so the wide columns can reduce early.
TILES_SMALL_FIRST = [(12288, 256)] + [(i * 2048, 2048) for i in range(6)]
TILES_SMALL_LAST = [(i * 2048, 2048) for i in range(6)] + [(12288, 256)]
CPAD = 12544
NPAD_TOT = (CPAD - CSH) * NCORES

M0 = 0.5
M_MIN = 0.25
SCALE = 32.0
SHIFT = 32.0               # fixed log-softmax shift (|logits| <= SCALE)
FP8_PRESCALE = 8.0         # both operands scaled by 8 -> matmul gives 64*cos

f32 = mybir.dt.float32
bf16 = mybir.dt.bfloat16
fp8 = mybir.dt.float8e4

NBC = B // 128             # 4 batch chunks

_cached_nc = None
_last_results = None


def _build():
    global _cached_nc
    if _cached_nc is not None:
        return _cached_nc

    nc = bacc.Bacc(
        "TRN2", target_bir_lowering=False, debug=False, num_devices=NCORES
    )

    # [p, j, c] with contraction index k = j*128 + p
    wnT_d = nc.dram_tensor("wnT", [128, 2, CPAD], fp8, kind="ExternalInput")
    featnT_d = nc.dram_tensor("featnT", [128, 2, B], fp8, kind="ExternalInput")
    out_d = nc.dram_tensor("out", [128, NBC], f32, kind="ExternalOutput")

    with tile.TileContext(nc) as tc:
        with (
            tc.tile_pool(name="persist", bufs=1) as persist,
            tc.tile_pool(name="epool", bufs=5) as epool,
            tc.tile_pool(name="psum", bufs=2, space="PSUM") as psum,
        ):
            fsb = persist.tile([128, 2, B], fp8)
            nc.sync.dma_start(out=fsb[:], in_=featnT_d[:])

            wsb = persist.tile([128, 2, CPAD], fp8)
            # chunked loads in consumption order: the small tile's columns
            # first (gates the pipeline start), then the wide region.
            # Spread across the three DMA-capable engines' queues — a single
            # queue serializes at ~50GB/s and trickles for the whole kernel.
            # (engine, lo, hi) in consumption order across the two HWDGE
            # queues (gpsimd's SWDGE path is too slow for bulk loads)
            plan = [
                (nc.scalar, 12288, 12544),
                (nc.scalar, 0, 1536),
                (nc.sync, 1536, 3072),
                (nc.scalar, 3072, 4608),
                (nc.sync, 4608, 6144),
                (nc.scalar, 6144, 7680),
                (nc.sync, 7680, 9216),
                (nc.scalar, 9216, 10752),
                (nc.sync, 10752, 12288),
            ]
            for eng, lo, hi in plan:
                eng.dma_start(
                    out=wsb[:, :, lo:hi], in_=wnT_d[:, :, lo:hi]
                )

            bias_s = persist.tile([128, 1], f32)
            nc.gpsimd.memset(bias_s[:], -SHIFT)

            eacc = [
                persist.tile(
                    [128, 2048], bf16, tag=f"eacc{bc}", name=f"eacc{bc}"
                )
                for bc in range(NBC)
            ]

            S_all = persist.tile([128, NBC], f32)

            S_main = persist.tile([128, 1], f32)

            for bc in range(NBC):
                last = bc == NBC - 1
                tiles = TILES_SMALL_LAST if last else TILES_SMALL_FIRST
                lhs = fsb[:, :, bc * 128:(bc + 1) * 128]
                for ti, (c0, cw) in enumerate(tiles):
                    ps = psum.tile([128, 2048], f32, tag="ps")
                    for j in range(0, cw, 512):
                        jw = min(512, cw - j)
                        nc.tensor.matmul(
                            ps[:, j:j + jw],
                            lhs,
                            wsb[:, :, c0 + j:c0 + j + jw],
                            start=True, stop=True,
                            perf_mode=mybir.MatmulPerfMode.DoubleRow,
                        )
                    esc = epool.tile([128, 2048], bf16, tag="esc")
                    nc.scalar.activation(
                        esc[:, :cw], ps[:, :cw],
                        mybir.ActivationFunctionType.Exp,
                        bias=bias_s[:], scale=SCALE / (FP8_PRESCALE**2),
                    )
                    if ti == 0:
                        nc.vector.tensor_copy(
                            eacc[bc][:, :cw], esc[:, :cw]
                        )
                    elif ti == 1 and not last:
                        nc.vector.tensor_add(
                            eacc[bc][:, :256], eacc[bc][:, :256], esc[:, :256]
                        )
                        nc.vector.tensor_copy(
                            eacc[bc][:, 256:], esc[:, 256:]
                        )
                    else:
                        nc.vector.tensor_add(
                            eacc[bc][:, :cw], eacc[bc][:, :cw], esc[:, :cw]
                        )
                    if bc > 0 and ti == 1:
                        nc.vector.tensor_reduce(
                            S_all[:, bc - 1:bc],
                            eacc[bc - 1][:],
                            axis=mybir.AxisListType.X,
                            op=mybir.AluOpType.add,
                        )
                    if last and ti == len(tiles) - 2:
                        nc.vector.tensor_reduce(
                            S_main[:],
                            eacc[bc][:, 256:2048],
                            axis=mybir.AxisListType.X,
                            op=mybir.AluOpType.add,
                        )

            S_small = persist.tile([128, 1], f32)
            nc.vector.tensor_reduce(
                S_small[:],
                eacc[NBC - 1][:, 0:256],
                axis=mybir.AxisListType.X,
                op=mybir.AluOpType.add,
            )
            nc.vector.tensor_add(S_all[:, NBC - 1:NBC], S_main[:], S_small[:])

            nc.sync.dma_start(out=out_d[:], in_=S_all[:])

    nc.compile()
    _cached_nc = nc
    return nc


def _host_prep(features, weight, weights, labels):
    """Everything O(B*D) / O(C*D) that is not the big matmul."""
    f = features.astype(np.float64)
    norms = np.sqrt((f * f).sum(axis=1))
    lo, hi = norms.min(), norms.max()
    denom = max(hi - lo, 1e-8)
    margins = np.clip(M_MIN + (M0 - M_MIN) * (norms - lo) / denom, M_MIN, M0)
    feat_n = f / np.maximum(norms, 1e-12)[:, None]

    wlab = weight[labels].astype(np.float64)
    wlab_n = wlab / np.maximum(
        np.sqrt((wlab * wlab).sum(axis=1)), 1e-12
    )[:, None]
    cos_t = np.clip((feat_n * wlab_n).sum(axis=1), -1.0 + 1e-7, 1.0 - 1e-7)
    cos_m = cos_t * np.cos(margins) - np.sqrt(1.0 - cos_t * cos_t) * np.sin(
        margins
    )
    t_logit = SCALE * cos_m
    corr = (
        np.exp(SCALE * cos_m - SHIFT)
        - np.exp(SCALE * cos_t - SHIFT)
        - NPAD_TOT * np.exp(-SHIFT)
    )
    coef = weights.astype(np.float64) / B
    return feat_n, corr, coef, t_logit


def _to_dr_layout(mat_t, width):
    """[D, X] f32 -> [128, 2, X] fp8 with k = j*128 + p."""
    a = mat_t.reshape(2, 128, width)          # [j, p, X]
    a = np.ascontiguousarray(a.transpose(1, 0, 2))  # [p, j, X]
    return a.astype(ml_dtypes.float8_e4m3)


def kernel(features, weight, weights, labels):
    global _last_results
    features = np.asarray(features, dtype=np.float32)
    weight = np.asarray(weight, dtype=np.float32)
    weights = np.asarray(weights, dtype=np.float32)
    labels = np.asarray(labels).astype(np.int64)

    feat_n, corr, coef, t_logit = _host_prep(features, weight, weights, labels)

    wn = weight / np.maximum(
        np.linalg.norm(weight, axis=1, keepdims=True), 1e-12
    )
    featnT = np.ascontiguousarray(feat_n.T.astype(np.float32)) * FP8_PRESCALE
    featnT8 = _to_dr_layout(featnT, B)

    in_maps = []
    for i in range(NCORES):
        sh = wn[i * CSH:(i + 1) * CSH]  # [CSH, D]
        wt = np.zeros((D, CPAD), dtype=np.float32)
        wt[:, :CSH] = sh.T * FP8_PRESCALE
        in_maps.append(
            {"wnT": _to_dr_layout(wt, CPAD), "featnT": featnT8}
        )

    nc = _build()
    res = run_bass_kernel_spmd(nc, in_maps, list(range(NCORES)))
    _last_results = res

    # ---- host combine ----
    S = np.zeros(B, dtype=np.float64)
    for i in range(NCORES):
        sc = np.asarray(res.results[i]["out"], dtype=np.float64)  # [128, 4]
        for bc in range(NBC):
            S[bc * 128:(bc + 1) * 128] += sc[:, bc]

    Z = S + corr
    per = SHIFT + np.log(Z) - t_logit
    loss = float((coef * per).sum())
    return np.array(loss, dtype=np.float32)
